# revision 8
# baseline (speedup 1.0000x reference)
"""Trainium2 Bass kernel for nn_DRNLayer (8-core n_upper-sharded).

out[i,j,l] = softmax_l( sum_k log( sum_m exp(w[j,k]*logD[m,l]) * P[i,k,m] ) + B[j,l] )

Sharding: n_upper (j) split 8 ways, 16 j per core. Softmax axis (q_up) is
local, so no collectives; host concatenates per-core outputs.
"""

import sys

sys.path.insert(0, "/opt/trn_rl_repo")

from contextlib import ExitStack

import numpy as np

import concourse.bacc as bacc
import concourse.bass as bass
import concourse.mybir as mybir
from concourse.bass_utils import run_bass_kernel_spmd
from concourse.tile import TileContext

F32 = mybir.dt.float32
F32R = mybir.dt.float32r
I32 = mybir.dt.int32
AF = mybir.ActivationFunctionType
ALU = mybir.AluOpType

N_CORES = 8
BATCH = 64  # i
NJ = 16  # j per core
NK = 128  # n_lower (k)
Q = 64  # q_upper == q_lower (l, m)
NKP = 64  # k-pairs: partition half 0 handles k=kp, half 1 handles k=kp+64
GS = 8  # k-pairs per product group (=> products of 8 fp32 values)
NG = NKP // GS  # 8 groups

_NC = None
LAST_RESULTS = None


def _build():
    nc = bacc.Bacc()
    P_d = nc.declare_dram_parameter("P", [BATCH, NK, Q], F32, isOutput=False)
    w_d = nc.declare_dram_parameter("warr", [2, NJ, NKP], F32, isOutput=False)
    b_d = nc.declare_dram_parameter("Bflat", [1, NJ * Q], F32, isOutput=False)
    o_d = nc.declare_dram_parameter("out", [BATCH, NJ, Q], F32, isOutput=True)

    with TileContext(nc) as tc, ExitStack() as ctx:
        consts = ctx.enter_context(tc.tile_pool(name="consts", bufs=1))
        ppool = ctx.enter_context(tc.tile_pool(name="pdata", bufs=1))
        tpool = ctx.enter_context(tc.tile_pool(name="tpair", bufs=3))
        apool = ctx.enter_context(tc.tile_pool(name="acc", bufs=2))
        gpool = ctx.enter_context(tc.tile_pool(name="glog", bufs=2))
        spool = ctx.enter_context(tc.tile_pool(name="smax", bufs=1))
        ps_tr = ctx.enter_context(tc.tile_pool(name="ptr", bufs=1, space="PSUM"))
        ps_pw = ctx.enter_context(tc.tile_pool(name="pw", bufs=2, space="PSUM"))
        ps_lg = ctx.enter_context(tc.tile_pool(name="lg", bufs=1, space="PSUM"))

        # ---------------- constants / inputs ----------------
        Pnat = ppool.tile([BATCH, NK, Q], F32)
        nc.sync.dma_start(out=Pnat, in_=P_d[:, :, :])

        # wball[64*par+m, j, kp] = w[j, kp + 64*par]  (replicated over m)
        wball = consts.tile([128, NJ, NKP], F32)
        for par in range(2):
            nc.sync.dma_start(
                out=wball[par * 64 : (par + 1) * 64, :, :],
                in_=w_d[par, :, :].unsqueeze(0).broadcast_to([64, NJ, NKP]),
            )

        Brow = consts.tile([1, NJ * Q], F32)
        nc.sync.dma_start(out=Brow, in_=b_d[:, :])

        # it[p, l] = l - (p % 64)
        it = consts.tile([128, Q], I32)
        nc.gpsimd.iota(it, pattern=[[1, Q]], base=0, channel_multiplier=-1)
        nc.vector.tensor_scalar_add(it[64:128, :], it[64:128, :], 64)
        # comb[p, i] = 1.0 if (p % 64) == i else 0  (cross-k-half combiner)
        comb = consts.tile([128, BATCH], F32)
        nc.vector.tensor_scalar(comb, it, 0, None, ALU.is_equal)
        # logDD[p, l] = -((l - m)/64)^2 , m = p % 64
        dd = consts.tile([128, Q], F32)
        nc.vector.tensor_scalar_mul(dd, it, 1.0 / Q)
        logDD = consts.tile([128, Q], F32)
        nc.vector.scalar_tensor_tensor(
            logDD, in0=dd, scalar=-1.0, in1=dd, op0=ALU.mult, op1=ALU.mult
        )
        # identity for PE transpose
        it2 = consts.tile([64, Q], I32)
        nc.gpsimd.iota(it2, pattern=[[1, Q]], base=0, channel_multiplier=-1)
        ident = consts.tile([64, Q], F32)
        nc.vector.tensor_scalar(ident, it2, 0, None, ALU.is_equal)
        ones64 = consts.tile([1, BATCH], F32)
        nc.vector.memset(ones64, 1.0)

        # ---------------- phase 1: transpose P ----------------
        # PT[64*par+m, kp, i] = P[i, kp + 64*par, m]
        # PE transpose must output at PSUM partition 0, so both k-halves are
        # transposed to partitions 0-63; the upper half is then moved to SBUF
        # partitions 64-127 by DMA (the only engine that can shift partitions).
        PTB = ppool.tile([128, NKP, 128], F32R)
        nc.gpsimd.memset(PTB[0:64, :, 64:128].bitcast(F32), 0.0)
        nc.gpsimd.memset(PTB[64:128, :, 0:64].bitcast(F32), 0.0)
        for kb in range(NKP // 8):
            pt_ps = ps_tr.tile([64, 16, BATCH], F32)  # 2 banks
            for s in range(8):
                kp = kb * 8 + s
                nc.tensor.transpose(pt_ps[:, s, :], Pnat[:, kp, :], ident)
                nc.tensor.transpose(pt_ps[:, 8 + s, :], Pnat[:, 64 + kp, :], ident)
            nc.vector.tensor_copy(
                out=PTB[0:64, kb * 8 : (kb + 1) * 8, 0:64], in_=pt_ps[:, 0:8, :]
            )
            stg = gpool.tile([64, 8, BATCH], F32R, tag="ptstage")
            nc.vector.tensor_copy(out=stg, in_=pt_ps[:, 8:16, :])
            nc.sync.dma_start(
                out=PTB[64:128, kb * 8 : (kb + 1) * 8, 64:128], in_=stg
            )

        # logits PSUM accumulator [i, jh, 512] (2 banks)
        logits = ps_lg.tile([BATCH, 2, 512], F32)

        # ---------------- phase 2: main loop over k-pairs ----------------
        acc = None
        for kp2 in range(NKP // 2):
            # T tile for two k-pairs: [128, q, j, l]
            arg = tpool.tile([128, 2, NJ, Q], F32, tag="arg")
            tp = tpool.tile([128, 2, NJ, Q], F32R, tag="texp")
            w_ap = (
                wball[:, :, 2 * kp2 : 2 * kp2 + 2]
                .rearrange("p j q -> p q j")
                .unsqueeze(3)
                .broadcast_to([128, 2, NJ, Q])
            )
            d_ap = (
                logDD[:, :].unsqueeze(1).unsqueeze(1).broadcast_to([128, 2, NJ, Q])
            )
            nc.gpsimd.tensor_tensor(out=arg, in0=w_ap, in1=d_ap, op=ALU.mult)
            nc.scalar.activation(out=tp, in_=arg, func=AF.Exp)

            for q in range(2):
                kp = 2 * kp2 + q
                g = kp // GS
                pw = ps_pw.tile([128, 2, 512], F32)  # 2 banks
                for jh in range(2):
                    nc.tensor.matmul(
                        out=pw[:, jh, :],
                        lhsT=PTB[:, kp, :],
                        rhs=tp[:, q, jh * 8 : (jh + 1) * 8, :].rearrange(
                            "p a b -> p (a b)"
                        ),
                        start=True,
                        stop=True,
                    )
                pw_flat = pw.rearrange("p a b -> p (a b)")
                if kp % GS == 0:
                    acc = apool.tile([128, NJ * Q], F32)
                    nc.vector.tensor_copy(out=acc, in_=pw_flat)
                else:
                    nc.vector.tensor_tensor(
                        out=acc, in0=pw_flat, in1=acc, op=ALU.mult
                    )
                if kp % GS == GS - 1:
                    gl = gpool.tile([128, NJ * Q], F32, tag="gl")
                    nc.scalar.activation(out=gl, in_=acc, func=AF.Ln)
                    for jh in range(2):
                        nc.tensor.matmul(
                            out=logits[:, jh, :],
                            lhsT=comb,
                            rhs=gl[:, jh * 512 : (jh + 1) * 512],
                            start=(g == 0),
                            stop=False,
                            skip_group_check=True,
                        )

        # bias: logits += 1 x Brow (broadcast over i)
        for jh in range(2):
            nc.tensor.matmul(
                out=logits[:, jh, :],
                lhsT=ones64,
                rhs=Brow[:, jh * 512 : (jh + 1) * 512],
                start=False,
                stop=True,
                skip_group_check=True,
            )

        # ---------------- phase 3: softmax over l ----------------
        lg_v = logits.rearrange("p a (j l) -> p (a j) l", l=Q)  # [64, 16, 64]
        mx = spool.tile([BATCH, NJ], F32)
        nc.vector.tensor_reduce(mx, lg_v, axis=mybir.AxisListType.X, op=ALU.max)
        em = spool.tile([BATCH, NJ, Q], F32)
        nc.vector.tensor_tensor(
            out=em,
            in0=lg_v,
            in1=mx.unsqueeze(2).broadcast_to([BATCH, NJ, Q]),
            op=ALU.subtract,
        )
        nc.scalar.activation(out=em, in_=em, func=AF.Exp)
        sm = spool.tile([BATCH, NJ], F32)
        nc.vector.tensor_reduce(sm, em, axis=mybir.AxisListType.X, op=ALU.add)
        rec = spool.tile([BATCH, NJ], F32)
        nc.vector.reciprocal(rec, sm)
        oute = spool.tile([BATCH, NJ, Q], F32)
        nc.gpsimd.tensor_tensor(
            out=oute,
            in0=em,
            in1=rec.unsqueeze(2).broadcast_to([BATCH, NJ, Q]),
            op=ALU.mult,
        )
        nc.sync.dma_start(out=o_d[:, :, :], in_=oute)

    nc.compile()
    return nc


def kernel(P, weight, bias_abs, bias_q, lambda_abs, lambda_q):
    global _NC, LAST_RESULTS
    P = np.ascontiguousarray(np.asarray(P, dtype=np.float32))
    weight = np.asarray(weight, dtype=np.float32)
    bias_abs = np.asarray(bias_abs, dtype=np.float32)
    bias_q = np.asarray(bias_q, dtype=np.float32)
    lambda_abs = np.asarray(lambda_abs, dtype=np.float32)
    lambda_q = np.asarray(lambda_q, dtype=np.float32)

    if _NC is None:
        _NC = _build()

    s = (np.arange(Q, dtype=np.float32) / Q)[None, :]  # [1, 64]
    in_maps = []
    for c in range(N_CORES):
        jsl = slice(c * NJ, (c + 1) * NJ)
        wsl = weight[jsl, :]  # [16, 128]
        warr = np.ascontiguousarray(wsl.reshape(NJ, 2, NKP).transpose(1, 0, 2))
        Bm = -bias_q[jsl] * (s - lambda_q[jsl]) ** 2 - bias_abs[jsl] * np.abs(
            s - lambda_abs[jsl]
        )  # [16, 64]
        in_maps.append(
            {
                "P": P,
                "warr": warr,
                "Bflat": np.ascontiguousarray(Bm.reshape(1, NJ * Q)),
            }
        )

    LAST_RESULTS = run_bass_kernel_spmd(_NC, in_maps, list(range(N_CORES)))
    return np.concatenate(
        [LAST_RESULTS.results[c]["out"] for c in range(N_CORES)], axis=1
    )
